# revision 2
# baseline (speedup 1.0000x reference)
"""Trainium2 Bass kernel for the CMDF block (dense_cnn) — v2.

Contract: kernel(**inputs) takes the FULL unsharded inputs (B=8, C=128,
H=W=64) and returns the FULL (8, 128, 64, 64) float32 output.
Sharding: data-parallel over batch — core b computes batch element b.

Math per batch element (see reference):
  Xs   = depthwise3x3(X2, static_w)
  ctx  = relu(w2 @ (w1 @ mean_hw([Xs; Y2])))
  cf   = (w3 @ ctx).reshape(C, 9)          # per-channel dynamic filter
  sf   = ws @ [Xs; Y2]                     # (9, H, W) spatial filter
  dyn  = sum_k shift_k(X2) * (cf[:, k] + sf[k])
  out  = wf[:, :C] @ Xs + wf[:, C:] @ dyn

v2 design notes (vs the f32r baseline):
  - Everything bf16 on the wide paths (halves DMA, enables DVE 2x modes).
  - mean_hw(Xs) is computed from X2 directly via boundary-corrected window
    sums (exact identity), so the context branch does NOT wait for Xs and
    phase C starts ~10us earlier.
  - The sf broadcast to 128 partitions moves off the PE: sf rows are staged
    to DRAM and broadcast back with stride-0-source pair-DMAs (1456ns per
    2 taps on the otherwise idle DMA engines); the k=8 tap is broadcast by
    a DVE stream_shuffle over the quadrant-replicated sfs layout.
  - The per-tap product (sf_bc + cf_k) * shift_k(X) is split across two
    routes to balance engines: DVE ts-add(4x) + tt-mult(2x), and Act
    bias-add + DVE tt-mult.
  - PE warm-up matmuls bridge the p-state ramp while input DMA completes.
"""

import numpy as np
import ml_dtypes

import concourse.bass as bass
import concourse.tile as tile
import concourse.mybir as mybir
from concourse.bass import AP
from concourse.bass_utils import run_bass_kernel_spmd

B, C, H, W, K = 8, 128, 64, 64, 3
HW = H * W
PH, PW = H + 2, W + 2          # 66x66 padded
NST = 4                        # super-tiles over rows
ROWS = H // NST                # 16 image rows per super-tile
STN = ROWS * W                 # 1024 px per super-tile
NT = K * K                     # 9 taps

F32 = mybir.dt.float32
BF16 = mybir.dt.bfloat16
ADD = mybir.AluOpType.add
SUB = mybir.AluOpType.subtract
MULT = mybir.AluOpType.mult
AX = mybir.AxisListType
ACT_COPY = mybir.ActivationFunctionType.Copy
ACT_IDENT = mybir.ActivationFunctionType.Identity
ACT_RELU = mybir.ActivationFunctionType.Relu

# ---- pk column layout (bf16 columns) ----
O_ID = 0                       # identity mask [C, C] bf16
O_SW = O_ID + C                # sw/HW [C, 9] f32 -> 18 bf cols
O_WSA = O_SW + 18              # wsaT [C, C] bf16 (quadrant-replicated)
O_WSB = O_WSA + C              # wsbT [C, C] bf16 (quadrant-replicated)
O_WFA = O_WSB + C              # wfaT [C, C] bf16
O_WFB = O_WFA + C              # wfbT [C, C] bf16
O_W21A = O_WFB + C             # (w2@w1)aT [C, 64] f32 -> 128 bf cols
O_W21B = O_W21A + 128          # (w2@w1)bT/HW [C, 64] f32 -> 128
O_W3 = O_W21B + 128            # w3T [64, 9*128] bf16 (parts 0-63)
O_XP = O_W3 + NT * C           # xpad [C, 66*66] bf16
O_Y2 = O_XP + PH * PW          # y2 [C, 4096] bf16
PK_COLS = O_Y2 + HW

# input DMA chunks (bf16 col ranges, in issue order)
XP_S1 = O_XP + 18 * PW         # xpad rows 0-17 (tile 0 + halo)
XP_S2 = O_XP + 34 * PW         # xpad rows 18-33 (tile 1 + halo)
Y2_SPLIT = O_Y2 + HW // 2
CHUNKS = [
    (O_ID, O_WSA),             # id + sw (for diag build + mean weights)
    (O_XP, XP_S1),             # xpad tile 0
    (XP_S1, XP_S2),            # xpad tile 1
    (O_Y2, Y2_SPLIT),          # y2 top
    (Y2_SPLIT, PK_COLS),       # y2 bottom
    (XP_S2, O_Y2),             # xpad bottom
    (O_WSA, O_XP),             # remaining weights
]

# ---- routing tables ----
# Broadcast groups: taps fetched in PAIR DMAs (halves serial HWDGE/issue
# overhead); queue 's' = SP (HWDGE), 'g' = gpsimd-issued (SWDGE runs on the
# Pool engine, bypasses HWDGE), 'h' = DVE stream_shuffle from the
# quadrant-replicated sfs tile (no DMA at all).
BCAST_GROUPS_T0 = None   # set to a list to special-case tile 0
BCAST_GROUPS = [((0, 1), 's'), ((2, 3), 's'), ((4, 5), 's'),
                ((6, 7), 's'), ((8,), 'h')]
# product route:
#   'v' = DVE stt (1 op, 1127ns)
#   'w' = DVE ts-add (4x, 330ns) + DVE tt-mult (2x, 594ns)
#   'a' = Act bias-add + DVE bf16 tt-mult
#   'g' = gpsimd stt
PROD_BY_K = {0: 'a', 1: 'w', 2: 'a', 3: 'w', 4: 'a',
             5: 'w', 6: 'a', 7: 'w', 8: 'w'}
PROD = [PROD_BY_K[k] for t in range(NST) for k in range(NT)]

N_WARMUP = 14                  # junk matmuls to ramp the PE p-state
PREFETCH_BCAST = False         # emit bcast DMAs in phase A vs phase C
RING_BC2, RING_BC1, RING_BCH, RING_P, RING_BCC = 6, 4, 4, 8, 6

_CACHE = {}


def _split_multiwaits(nc):
    """walrus codegen accepts only ONE embedded sem wait per instruction.
    Hoist excess waits onto same-engine NoOps placed immediately before."""
    ctr = 0
    for fn in nc.m.functions:
        for blk in fn.blocks:
            out = []
            for inst in blk.instructions:
                si = inst.sync_info
                waits = list(si.on_wait) if si is not None and si.on_wait else []
                # StreamShuffle / gpsimd ISA ops: codegen cannot embed ANY
                # sem wait on them — hoist all of them.
                limit = 0 if (isinstance(inst, mybir.InstStreamShuffle)
                              or type(inst).__name__ in
                              ("InstPartitionBroadcast", "InstISA")) else 1
                if len(waits) > limit:
                    keep = waits[-limit:] if limit else []
                    for w in waits[:len(waits) - limit]:
                        ctr += 1
                        out.append(mybir.InstNoOp(
                            name=f"I-wsplit-{ctr}",
                            engine=inst.engine,
                            ins=[], outs=[],
                            sync_info=mybir.SyncInfo(on_wait=[w], on_update=[]),
                        ))
                    inst.sync_info = mybir.SyncInfo(
                        on_wait=keep,
                        on_update=list(si.on_update) if si.on_update else [],
                    )
                out.append(inst)
            blk.instructions = out


def _build_bass():
    nc = bass.Bass("TRN2", target_bir_lowering=False, debug=False)

    pk = nc.dram_tensor("pk", [C, PK_COLS], BF16, kind="ExternalInput").ap()
    sfd = nc.dram_tensor("sfd", [NT, HW], BF16, kind="Internal").ap()
    ob = nc.dram_tensor("ob", [C, H, W], BF16, kind="ExternalOutput").ap()

    with tile.TileContext(nc) as tc:
        with tc.tile_pool(name="S", bufs=1) as S, \
             tc.tile_pool(name="pBC", bufs=8) as pBC, \
             tc.tile_pool(name="pBCC", bufs=RING_BCC) as pBCC, \
             tc.tile_pool(name="pP", bufs=RING_P) as pP, \
             tc.tile_pool(name="pOsb", bufs=3) as pOsb, \
             tc.tile_pool(name="psXS", bufs=1, space="PSUM") as psXS, \
             tc.tile_pool(name="psSF", bufs=1, space="PSUM") as psSF, \
             tc.tile_pool(name="psOUT", bufs=2, space="PSUM") as psOUT:

            stg = S.tile([C, PK_COLS], BF16)
            idm = stg[:, O_ID:O_ID + C]
            swh = stg[:, O_SW:O_SW + 18].bitcast(F32)          # [C, 9] f32
            wsaT = stg[:, O_WSA:O_WSA + C]
            wsbT = stg[:, O_WSB:O_WSB + C]
            wfaT = stg[:, O_WFA:O_WFA + C]
            wfbT = stg[:, O_WFB:O_WFB + C]
            w21aT = stg[:, O_W21A:O_W21A + 128].bitcast(F32)   # [C, 64]
            w21bT = stg[:, O_W21B:O_W21B + 128].bitcast(F32)
            w3T = stg[0:64, O_W3:O_W3 + NT * C]                # [64, 1152] bf16
            xpad = stg[:, O_XP:O_XP + PH * PW].rearrange(
                "p (h w) -> p h w", w=PW)
            y2 = stg[:, O_Y2:O_Y2 + HW]

            dsw = S.tile([C, NT * C], BF16)      # diag depthwise weights
            xs = S.tile([C, HW], BF16)           # Xs (depthwise out)
            # spatial filter rows, replicated per 32-partition quadrant
            # (row 32g+k = sf[k]) so stream_shuffle can broadcast any tap
            sfs = S.tile([C, HW], BF16)

            # mean-path scratch (f32)
            sparts = S.tile([C, 4], F32)         # X2 partial sums
            rsum = S.tile([C, 2], F32)           # row1 / row64 sums
            csum = S.tile([C, 2], F32)           # col1 / col64 sums
            y2s = S.tile([C, 2], F32)            # Y2 partial sums
            wsum = S.tile([C, NT], F32)          # 9 window sums
            stot = S.tile([C, 1], F32)
            mxs = S.tile([C, 1], F32)
            my2 = S.tile([C, 1], F32)
            ctx2 = S.tile([64, 1], BF16)
            cfsb = S.tile([C, NT], F32)
            junk = S.tile([C, 2248], BF16)

            # ---------------- input DMA ----------------
            for a, b in CHUNKS:
                nc.sync.dma_start(out=stg[:, a:b], in_=pk[:, a:b])

            # ---------------- PE warm-up ----------------
            warm = psOUT.tile([C, 512], F32, tag="ctx", bufs=1)
            for i in range(N_WARMUP):
                nc.tensor.matmul(warm[:, 0:128], idm, idm,
                                 start=(i == 0), stop=(i == N_WARMUP - 1))

            # ---------------- diag weight build (DVE) ----------------
            for k in range(NT):
                nc.vector.tensor_scalar(
                    out=dsw[:, k * C:(k + 1) * C], in0=idm,
                    scalar1=swh[:, k:k + 1], scalar2=float(HW), op0=MULT,
                    op1=MULT)  # sw/HW * HW = sw

            # ---------------- phase A, pipelined per super-tile ----
            bcv = {}   # (t, k) -> broadcast tile view for the products

            def phase_a(t):
                for h in range(2):
                    # separate 1-bank PSUM tile per half: the accumulation
                    # group's readiness is tile-granular, so a shared
                    # [C,2,512] tile would stall the h0 copy on h1's matmuls
                    xs_ps = psXS.tile([C, 512], F32, tag="xs", bufs=2)
                    for k in range(NT):
                        dh, dw = divmod(k, 3)
                        r0 = 16 * t + 8 * h + dh
                        nc.tensor.matmul(
                            xs_ps,
                            dsw[:, k * C:(k + 1) * C],
                            xpad[:, r0:r0 + 8, dw:dw + W],
                            start=(k == 0), stop=(k == NT - 1))
                    c0 = t * STN + h * 512
                    nc.scalar.copy(out=xs[:, c0:c0 + 512], in_=xs_ps)
                for h in range(2):
                    c0 = t * STN + h * 512
                    sf_ps = psSF.tile([C, 512], F32, tag="sf")
                    nc.tensor.matmul(sf_ps, wsaT, xs[:, c0:c0 + 512],
                                     start=True, stop=False)
                    nc.tensor.matmul(sf_ps, wsbT, y2[:, c0:c0 + 512],
                                     start=False, stop=True)
                    nc.scalar.copy(out=sfs[:, c0:c0 + 512], in_=sf_ps)
                # stage the tile's sf rows (quadrant 0) to DRAM for
                # broadcast-back
                nc.sync.dma_start(out=sfd[:, t * STN:(t + 1) * STN],
                                  in_=sfs[0:NT, t * STN:(t + 1) * STN])
                if PREFETCH_BCAST:
                    emit_bcasts(t)

            def emit_bcasts(t):
                # the pair DMAs ride the same SP queue as the sfd write
                # (FIFO gives the RAW ordering on DRAM)
                for ks, q in (BCAST_GROUPS_T0 if (t == 0 and BCAST_GROUPS_T0) else BCAST_GROUPS):
                    n = len(ks)
                    if q == 'h':
                        for k in ks:
                            bct = pBC.tile([C, ROWS, W], BF16,
                                           tag="bch", bufs=RING_BCH)
                            nc.vector.stream_shuffle(
                                bct.rearrange("p a b -> p (a b)"),
                                sfs[:, t * STN:(t + 1) * STN], [k] * 32)
                            bcv[(t, k)] = bct
                        continue
                    bct = pBC.tile([C, n, ROWS, W], BF16,
                                   tag=f"bc{n}", bufs=(RING_BC2 if n == 2 else RING_BC1))
                    src = AP(sfd.tensor, ks[0] * HW + t * STN,
                             [[0, C], [HW, n], [1, STN]])
                    dma = nc.sync.dma_start if q == 's' else nc.gpsimd.dma_start
                    dma(out=bct.rearrange("p a b c -> p a (b c)"), in_=src)
                    for j, k in enumerate(ks):
                        bcv[(t, k)] = bct[:, j]

            phase_a(0)

            # ------- mean path (from X2/Y2 directly), readiness order ----
            nc.vector.tensor_scalar(
                out=junk[:, 0:XP_S1 - O_XP], in0=stg[:, O_XP:XP_S1],
                scalar1=0.0, scalar2=1.0, op0=ADD, op1=MULT,
                accum_out=sparts[:, 0:1])
            nc.vector.tensor_reduce(out=rsum[:, 0:1], in_=xpad[:, 1, :],
                                    axis=AX.X, op=ADD)
            nc.vector.tensor_scalar(
                out=junk[:, 0:XP_S2 - XP_S1], in0=stg[:, XP_S1:XP_S2],
                scalar1=0.0, scalar2=1.0, op0=ADD, op1=MULT,
                accum_out=sparts[:, 1:2])
            nc.vector.tensor_scalar(
                out=junk[:, 0:HW // 2], in0=stg[:, O_Y2:Y2_SPLIT],
                scalar1=0.0, scalar2=1.0, op0=ADD, op1=MULT,
                accum_out=y2s[:, 0:1])
            nc.vector.tensor_scalar(
                out=junk[:, 0:HW // 2], in0=stg[:, Y2_SPLIT:PK_COLS],
                scalar1=0.0, scalar2=1.0, op0=ADD, op1=MULT,
                accum_out=y2s[:, 1:2])
            nc.vector.tensor_scalar(
                out=junk[:, 0:O_Y2 - XP_S2], in0=stg[:, XP_S2:O_Y2],
                scalar1=0.0, scalar2=1.0, op0=ADD, op1=MULT,
                accum_out=sparts[:, 2:3])
            nc.vector.tensor_reduce(out=rsum[:, 1:2], in_=xpad[:, 64, :],
                                    axis=AX.X, op=ADD)
            nc.vector.tensor_reduce(out=csum[:, 0:1],
                                    in_=xpad[:, :, 1:2].rearrange("p a b -> p (a b)"),
                                    axis=AX.X, op=ADD)
            nc.vector.tensor_reduce(out=csum[:, 1:2],
                                    in_=xpad[:, :, 64:65].rearrange("p a b -> p (a b)"),
                                    axis=AX.X, op=ADD)
            nc.vector.tensor_reduce(out=stot, in_=sparts[:, 0:3], axis=AX.X, op=ADD)
            # window sums: wsum[c, 3a+b]
            nc.vector.memset(wsum, 0.0)
            nc.vector.tensor_scalar(out=wsum, in0=wsum, scalar1=stot,
                                    scalar2=None, op0=ADD)
            w3d = wsum.rearrange("p (a b) -> p a b", b=3)
            nc.vector.tensor_scalar(out=w3d[:, 0, :], in0=w3d[:, 0, :],
                                    scalar1=rsum[:, 1:2], scalar2=None, op0=SUB)
            nc.vector.tensor_scalar(out=w3d[:, 2, :], in0=w3d[:, 2, :],
                                    scalar1=rsum[:, 0:1], scalar2=None, op0=SUB)
            nc.vector.tensor_scalar(out=w3d[:, :, 0:1].rearrange("p a b -> p (a b)"),
                                    in0=w3d[:, :, 0:1].rearrange("p a b -> p (a b)"),
                                    scalar1=csum[:, 1:2], scalar2=None, op0=SUB)
            nc.vector.tensor_scalar(out=w3d[:, :, 2:3].rearrange("p a b -> p (a b)"),
                                    in0=w3d[:, :, 2:3].rearrange("p a b -> p (a b)"),
                                    scalar1=csum[:, 0:1], scalar2=None, op0=SUB)
            # corner add-backs: (a,b)=(0,0)->X[64,64], (0,2)->X[64,1],
            # (2,0)->X[1,64], (2,2)->X[1,1]  (padded coords)
            for (kk, rr, cc) in ((0, 64, 64), (2, 64, 1), (6, 1, 64), (8, 1, 1)):
                nc.vector.tensor_tensor(out=wsum[:, kk:kk + 1],
                                        in0=wsum[:, kk:kk + 1],
                                        in1=xpad[:, rr, cc:cc + 1], op=ADD)
            nc.vector.tensor_tensor(out=wsum, in0=wsum, in1=swh, op=MULT)
            nc.vector.tensor_reduce(out=mxs, in_=wsum, axis=AX.X, op=ADD)
            nc.vector.tensor_reduce(out=my2, in_=y2s, axis=AX.X, op=ADD)

            # ---------------- context branch (w2@w1 folded on host) -----
            ctxp = psOUT.tile([C, 16], F32, tag="ctx", bufs=1)
            nc.tensor.matmul(ctxp[0:64, 0:1], w21aT, mxs, start=True, stop=False)
            nc.tensor.matmul(ctxp[0:64, 0:1], w21bT, my2, start=False, stop=True)
            nc.scalar.activation(out=ctx2, in_=ctxp[0:64, 0:1], func=ACT_RELU)
            for k in range(NT):
                nc.tensor.matmul(ctxp[:, 2 + k:3 + k],
                                 w3T[:, k * C:(k + 1) * C], ctx2,
                                 start=True, stop=True)
            nc.scalar.copy(out=cfsb, in_=ctxp[:, 2:2 + NT])

            phase_a(1)
            phase_a(2)
            phase_a(3)

            # ---------------- phase C ----------------
            for t in range(NST):
                out_ps = psOUT.tile([C, 2, 512], F32, tag="out")
                for h in range(2):
                    c0 = t * STN + h * 512
                    nc.tensor.matmul(out_ps[:, h], wfaT, xs[:, c0:c0 + 512],
                                     start=True, stop=False)
                if not PREFETCH_BCAST:
                    emit_bcasts(t)
                for k in range(NT):
                    slot = t * NT + k
                    dh, dw = divmod(k, 3)
                    bc = bcv[(t, k)]
                    xv = xpad[:, 16 * t + dh:16 * t + dh + ROWS, dw:dw + W]
                    p_sb = pP.tile([C, ROWS, W], BF16, tag="p")
                    r = PROD[slot]
                    if r == 'v':
                        nc.vector.scalar_tensor_tensor(
                            out=p_sb, in0=bc, scalar=cfsb[:, k:k + 1],
                            in1=xv, op0=ADD, op1=MULT)
                    elif r == 'g':
                        nc.gpsimd.scalar_tensor_tensor(
                            out=p_sb, in0=bc, scalar=cfsb[:, k:k + 1],
                            in1=xv, op0=ADD, op1=MULT)
                    elif r == 'w':
                        bcc = pBCC.tile([C, ROWS, W], BF16, tag="bcc")
                        nc.vector.tensor_scalar(
                            out=bcc, in0=bc, scalar1=cfsb[:, k:k + 1],
                            scalar2=None, op0=ADD)
                        nc.vector.tensor_tensor(out=p_sb, in0=bcc, in1=xv,
                                                op=MULT)
                    else:
                        bcc = pBCC.tile([C, ROWS, W], BF16, tag="bcc")
                        nc.scalar.activation(out=bcc, in_=bc, func=ACT_IDENT,
                                             bias=cfsb[:, k:k + 1])
                        nc.vector.tensor_tensor(out=p_sb, in0=bcc, in1=xv,
                                                op=MULT)
                    pv = p_sb.rearrange("p a b -> p (a b)")
                    for h in range(2):
                        nc.tensor.matmul(out_ps[:, h], wfbT,
                                         pv[:, h * 512:(h + 1) * 512],
                                         start=False,
                                         stop=(k == NT - 1))
                # per-half output copy on the Act queue (keeps the SP queue
                # free for the broadcast stream); the last tile ships each
                # half separately to shorten the tail
                o_sb = pOsb.tile([C, 2, 8, W], BF16, tag="osb")
                for h in range(2):
                    nc.scalar.copy(out=o_sb[:, h], in_=out_ps[:, h].rearrange(
                        "c (r w) -> c r w", w=W))
                nc.scalar.dma_start(
                    out=ob[:, 16 * t:16 * t + 16, :],
                    in_=o_sb.rearrange("c b r w -> c (b r) w"))

    _split_multiwaits(nc)
    return nc


def _bf(a):
    return np.ascontiguousarray(a, dtype=np.float32).astype(ml_dtypes.bfloat16)


def _f32cols(a):
    """f32 [P, n] -> bf16-typed [P, 2n] view of the same bytes."""
    a = np.ascontiguousarray(a, dtype=np.float32)
    return a.view(np.uint16).view(ml_dtypes.bfloat16)


def _prep_weights(static_w, w1, w2, w3, ws, wf):
    f = np.float32
    pk_w = np.zeros((C, O_XP), dtype=ml_dtypes.bfloat16)
    pk_w[:, O_ID:O_ID + C] = _bf(np.eye(C, dtype=f))
    sw = np.ascontiguousarray(static_w.reshape(C, NT), dtype=f) / HW
    pk_w[:, O_SW:O_SW + 18] = _f32cols(sw)
    wsa_rep = np.zeros((C, C), dtype=f)
    wsb_rep = np.zeros((C, C), dtype=f)
    for g in range(4):
        wsa_rep[:, 32 * g:32 * g + NT] = ws[:, :C].T
        wsb_rep[:, 32 * g:32 * g + NT] = ws[:, C:].T
    pk_w[:, O_WSA:O_WSA + C] = _bf(wsa_rep)
    pk_w[:, O_WSB:O_WSB + C] = _bf(wsb_rep)
    pk_w[:, O_WFA:O_WFA + C] = _bf(wf[:, :C].T)
    pk_w[:, O_WFB:O_WFB + C] = _bf(wf[:, C:].T)
    w21 = (w2.astype(f) @ w1.astype(f))           # fold: relu((w2@w1)@mean)
    pk_w[:, O_W21A:O_W21A + 128] = _f32cols(w21[:, :C].T)
    pk_w[:, O_W21B:O_W21B + 128] = _f32cols(w21[:, C:].T / HW)
    w3t = np.ascontiguousarray(
        w3.reshape(C, NT, 64).transpose(2, 1, 0), dtype=f).reshape(64, NT * C)
    pk_w[0:64, O_W3:O_W3 + NT * C] = _bf(w3t)
    return pk_w


def make_in_maps(X2, Y2, static_w, w1, w2, w3, ws, wf):
    pk_w = _prep_weights(
        np.asarray(static_w), np.asarray(w1), np.asarray(w2),
        np.asarray(w3), np.asarray(ws), np.asarray(wf))
    X2 = np.asarray(X2)
    Y2 = np.asarray(Y2)
    xpad_all = np.zeros((B, C, PH, PW), dtype=np.float32)
    xpad_all[:, :, 1:H + 1, 1:W + 1] = X2
    xpad_all = _bf(xpad_all).reshape(B, C, PH * PW)
    y2_all = _bf(Y2.reshape(B, C, HW))
    in_maps = []
    for b in range(B):
        m = {"pk": np.ascontiguousarray(np.concatenate(
            [pk_w, xpad_all[b], y2_all[b]], axis=1))}
        in_maps.append(m)
    return in_maps


def get_nc():
    if "nc" not in _CACHE:
        _CACHE["nc"] = _build_bass()
    return _CACHE["nc"]


def kernel(X2, Y2, static_w, w1, w2, w3, ws, wf):
    nc = get_nc()
    in_maps = make_in_maps(
        np.asarray(X2), np.asarray(Y2), static_w, w1, w2, w3, ws, wf)
    res = run_bass_kernel_spmd(nc, in_maps, core_ids=list(range(B)))
    out = np.stack([r["ob"] for r in res.results]).astype(np.float32)
    return out


# revision 3
# speedup vs baseline: 1.0623x; 1.0623x over previous
"""Trainium2 Bass kernel for the CMDF block (dense_cnn) — v2.

Contract: kernel(**inputs) takes the FULL unsharded inputs (B=8, C=128,
H=W=64) and returns the FULL (8, 128, 64, 64) float32 output.
Sharding: data-parallel over batch — core b computes batch element b.

Math per batch element (see reference):
  Xs   = depthwise3x3(X2, static_w)
  ctx  = relu(w2 @ (w1 @ mean_hw([Xs; Y2])))
  cf   = (w3 @ ctx).reshape(C, 9)          # per-channel dynamic filter
  sf   = ws @ [Xs; Y2]                     # (9, H, W) spatial filter
  dyn  = sum_k shift_k(X2) * (cf[:, k] + sf[k])
  out  = wf[:, :C] @ Xs + wf[:, C:] @ dyn

v2 design notes (vs the f32r baseline):
  - Everything bf16 on the wide paths (halves DMA, enables DVE 2x modes).
  - mean_hw(Xs) is computed from X2 directly via boundary-corrected window
    sums (exact identity), so the context branch does NOT wait for Xs and
    phase C starts ~10us earlier.
  - The sf broadcast to 128 partitions moves off the PE: sf rows are staged
    to DRAM and broadcast back with stride-0-source pair-DMAs (1456ns per
    2 taps on the otherwise idle DMA engines); the k=8 tap is broadcast by
    a DVE stream_shuffle over the quadrant-replicated sfs layout.
  - The per-tap product (sf_bc + cf_k) * shift_k(X) is split across two
    routes to balance engines: DVE ts-add(4x) + tt-mult(2x), and Act
    bias-add + DVE tt-mult.
  - PE warm-up matmuls bridge the p-state ramp while input DMA completes.
"""

import numpy as np
import ml_dtypes

import concourse.bass as bass
import concourse.tile as tile
import concourse.mybir as mybir
from concourse.bass import AP
from concourse.bass_utils import run_bass_kernel_spmd

B, C, H, W, K = 8, 128, 64, 64, 3
HW = H * W
PH, PW = H + 2, W + 2          # 66x66 padded
NST = 4                        # super-tiles over rows
ROWS = H // NST                # 16 image rows per super-tile
STN = ROWS * W                 # 1024 px per super-tile
NT = K * K                     # 9 taps

F32 = mybir.dt.float32
BF16 = mybir.dt.bfloat16
ADD = mybir.AluOpType.add
SUB = mybir.AluOpType.subtract
MULT = mybir.AluOpType.mult
AX = mybir.AxisListType
ACT_COPY = mybir.ActivationFunctionType.Copy
ACT_IDENT = mybir.ActivationFunctionType.Identity
ACT_RELU = mybir.ActivationFunctionType.Relu

# ---- pk column layout (bf16 columns) ----
O_ID = 0                       # identity mask [C, C] bf16
O_SW = O_ID + C                # sw/HW [C, 9] f32 -> 18 bf cols
O_WSA = O_SW + 18              # wsaT [C, C] bf16 (quadrant-replicated)
O_WSB = O_WSA + C              # wsbT [C, C] bf16 (quadrant-replicated)
O_WFA = O_WSB + C              # wfaT [C, C] bf16
O_WFB = O_WFA + C              # wfbT [C, C] bf16
O_W21A = O_WFB + C             # (w2@w1)aT [C, 64] f32 -> 128 bf cols
O_W21B = O_W21A + 128          # (w2@w1)bT/HW [C, 64] f32 -> 128
O_W3 = O_W21B + 128            # w3T [64, 9*128] bf16 (parts 0-63)
O_XP = O_W3 + NT * C           # xpad [C, 66*66] bf16
O_Y2 = O_XP + PH * PW          # y2 [C, 4096] bf16
PK_COLS = O_Y2 + HW

# input DMA chunks (bf16 col ranges, in issue order)
XP_S1 = O_XP + 18 * PW         # xpad rows 0-17 (tile 0 + halo)
XP_S2 = O_XP + 34 * PW         # xpad rows 18-33 (tile 1 + halo)
Y2_SPLIT = O_Y2 + HW // 2
CHUNKS = [
    (O_ID, O_WSA),             # id + sw (for diag build + mean weights)
    (O_XP, XP_S1),             # xpad tile 0
    (XP_S1, XP_S2),            # xpad tile 1
    (O_Y2, Y2_SPLIT),          # y2 top
    (Y2_SPLIT, PK_COLS),       # y2 bottom
    (XP_S2, O_Y2),             # xpad bottom
    (O_WSA, O_XP),             # remaining weights
]

# ---- routing tables ----
# Broadcast groups: taps fetched in PAIR DMAs (halves serial HWDGE/issue
# overhead); queue 's' = SP (HWDGE), 'g' = gpsimd-issued (SWDGE runs on the
# Pool engine, bypasses HWDGE), 'h' = DVE stream_shuffle from the
# quadrant-replicated sfs tile (no DMA at all).
# tile 0 bridges the DRAM-roundtrip latency with DVE shuffles for its
# first taps (phase C can start as soon as sfs lands in SBUF)
BCAST_GROUPS_T0 = [((0,), 'h'), ((1,), 'h'), ((2,), 'h'), ((3, 4), 's'),
                   ((5, 6), 's'), ((7, 8), 's')]
BCAST_GROUPS = [((0, 1), 's'), ((2, 3), 's'), ((4, 5), 's'),
                ((6, 7), 's'), ((8,), 'h')]
# product route:
#   'v' = DVE stt (1 op, 1127ns)
#   'w' = DVE ts-add (4x, 330ns) + DVE tt-mult (2x, 594ns)
#   'a' = Act bias-add + DVE bf16 tt-mult
#   'g' = gpsimd stt
PROD_BY_K = {0: 'a', 1: 'w', 2: 'a', 3: 'w', 4: 'a',
             5: 'w', 6: 'a', 7: 'w', 8: 'a'}
PROD = [PROD_BY_K[k] for t in range(NST) for k in range(NT)]

N_WARMUP = 14                  # junk matmuls to ramp the PE p-state
PREFETCH_BCAST = False         # emit bcast DMAs in phase A vs phase C
RING_BC2, RING_BC1, RING_BCH, RING_P, RING_BCC = 6, 4, 4, 8, 6

_CACHE = {}


def _split_multiwaits(nc):
    """walrus codegen accepts only ONE embedded sem wait per instruction.
    Hoist excess waits onto same-engine NoOps placed immediately before."""
    ctr = 0
    for fn in nc.m.functions:
        for blk in fn.blocks:
            out = []
            for inst in blk.instructions:
                si = inst.sync_info
                waits = list(si.on_wait) if si is not None and si.on_wait else []
                # StreamShuffle / gpsimd ISA ops: codegen cannot embed ANY
                # sem wait on them — hoist all of them.
                limit = 0 if (isinstance(inst, mybir.InstStreamShuffle)
                              or type(inst).__name__ in
                              ("InstPartitionBroadcast", "InstISA")) else 1
                if len(waits) > limit:
                    keep = waits[-limit:] if limit else []
                    for w in waits[:len(waits) - limit]:
                        ctr += 1
                        out.append(mybir.InstNoOp(
                            name=f"I-wsplit-{ctr}",
                            engine=inst.engine,
                            ins=[], outs=[],
                            sync_info=mybir.SyncInfo(on_wait=[w], on_update=[]),
                        ))
                    inst.sync_info = mybir.SyncInfo(
                        on_wait=keep,
                        on_update=list(si.on_update) if si.on_update else [],
                    )
                out.append(inst)
            blk.instructions = out


def _build_bass():
    nc = bass.Bass("TRN2", target_bir_lowering=False, debug=False)

    pk = nc.dram_tensor("pk", [C, PK_COLS], BF16, kind="ExternalInput").ap()
    sfd = nc.dram_tensor("sfd", [NT, HW], BF16, kind="Internal").ap()
    ob = nc.dram_tensor("ob", [C, H, W], BF16, kind="ExternalOutput").ap()

    with tile.TileContext(nc) as tc:
        with tc.tile_pool(name="S", bufs=1) as S, \
             tc.tile_pool(name="pBC", bufs=8) as pBC, \
             tc.tile_pool(name="pBCC", bufs=RING_BCC) as pBCC, \
             tc.tile_pool(name="pP", bufs=RING_P) as pP, \
             tc.tile_pool(name="pOsb", bufs=3) as pOsb, \
             tc.tile_pool(name="psXS", bufs=1, space="PSUM") as psXS, \
             tc.tile_pool(name="psSF", bufs=1, space="PSUM") as psSF, \
             tc.tile_pool(name="psOUT", bufs=2, space="PSUM") as psOUT:

            stg = S.tile([C, PK_COLS], BF16)
            idm = stg[:, O_ID:O_ID + C]
            swh = stg[:, O_SW:O_SW + 18].bitcast(F32)          # [C, 9] f32
            wsaT = stg[:, O_WSA:O_WSA + C]
            wsbT = stg[:, O_WSB:O_WSB + C]
            wfaT = stg[:, O_WFA:O_WFA + C]
            wfbT = stg[:, O_WFB:O_WFB + C]
            w21aT = stg[:, O_W21A:O_W21A + 128].bitcast(F32)   # [C, 64]
            w21bT = stg[:, O_W21B:O_W21B + 128].bitcast(F32)
            w3T = stg[0:64, O_W3:O_W3 + NT * C]                # [64, 1152] bf16
            xpad = stg[:, O_XP:O_XP + PH * PW].rearrange(
                "p (h w) -> p h w", w=PW)
            y2 = stg[:, O_Y2:O_Y2 + HW]

            dsw = S.tile([C, NT * C], BF16)      # diag depthwise weights
            xs = S.tile([C, HW], BF16)           # Xs (depthwise out)
            # spatial filter rows, replicated per 32-partition quadrant
            # (row 32g+k = sf[k]) so stream_shuffle can broadcast any tap
            sfs = S.tile([C, HW], BF16)

            # mean-path scratch (f32)
            sparts = S.tile([C, 4], F32)         # X2 partial sums
            rsum = S.tile([C, 2], F32)           # row1 / row64 sums
            csum = S.tile([C, 2], F32)           # col1 / col64 sums
            y2s = S.tile([C, 2], F32)            # Y2 partial sums
            wsum = S.tile([C, NT], F32)          # 9 window sums
            stot = S.tile([C, 1], F32)
            mxs = S.tile([C, 1], F32)
            my2 = S.tile([C, 1], F32)
            ctx2 = S.tile([64, 1], BF16)
            cfsb = S.tile([C, NT], F32)
            junk = S.tile([C, 2248], BF16)

            # ---------------- input DMA ----------------
            for a, b in CHUNKS:
                nc.sync.dma_start(out=stg[:, a:b], in_=pk[:, a:b])

            # ---------------- PE warm-up ----------------
            warm = psOUT.tile([C, 512], F32, tag="ctx", bufs=1)
            for i in range(N_WARMUP):
                nc.tensor.matmul(warm[:, 0:128], idm, idm,
                                 start=(i == 0), stop=(i == N_WARMUP - 1))

            # ---------------- diag weight build (DVE) ----------------
            for k in range(NT):
                nc.vector.tensor_scalar(
                    out=dsw[:, k * C:(k + 1) * C], in0=idm,
                    scalar1=swh[:, k:k + 1], scalar2=float(HW), op0=MULT,
                    op1=MULT)  # sw/HW * HW = sw

            # ---------------- phase A, pipelined per super-tile ----
            bcv = {}   # (t, k) -> broadcast tile view for the products

            def phase_a(t):
                for h in range(2):
                    # separate 1-bank PSUM tile per half: the accumulation
                    # group's readiness is tile-granular, so a shared
                    # [C,2,512] tile would stall the h0 copy on h1's matmuls
                    xs_ps = psXS.tile([C, 512], F32, tag="xs", bufs=2)
                    for k in range(NT):
                        dh, dw = divmod(k, 3)
                        r0 = 16 * t + 8 * h + dh
                        nc.tensor.matmul(
                            xs_ps,
                            dsw[:, k * C:(k + 1) * C],
                            xpad[:, r0:r0 + 8, dw:dw + W],
                            start=(k == 0), stop=(k == NT - 1))
                    c0 = t * STN + h * 512
                    nc.scalar.copy(out=xs[:, c0:c0 + 512], in_=xs_ps)
                for h in range(2):
                    c0 = t * STN + h * 512
                    sf_ps = psSF.tile([C, 512], F32, tag="sf")
                    nc.tensor.matmul(sf_ps, wsaT, xs[:, c0:c0 + 512],
                                     start=True, stop=False)
                    nc.tensor.matmul(sf_ps, wsbT, y2[:, c0:c0 + 512],
                                     start=False, stop=True)
                    nc.scalar.copy(out=sfs[:, c0:c0 + 512], in_=sf_ps)
                # stage the tile's sf rows (quadrant 0) to DRAM for
                # broadcast-back
                nc.sync.dma_start(out=sfd[:, t * STN:(t + 1) * STN],
                                  in_=sfs[0:NT, t * STN:(t + 1) * STN])
                if PREFETCH_BCAST:
                    emit_bcasts(t)

            def emit_bcasts(t):
                # the pair DMAs ride the same SP queue as the sfd write
                # (FIFO gives the RAW ordering on DRAM)
                for ks, q in (BCAST_GROUPS_T0 if (t == 0 and BCAST_GROUPS_T0) else BCAST_GROUPS):
                    n = len(ks)
                    if q == 'h':
                        for k in ks:
                            bct = pBC.tile([C, ROWS, W], BF16,
                                           tag="bch", bufs=RING_BCH)
                            nc.vector.stream_shuffle(
                                bct.rearrange("p a b -> p (a b)"),
                                sfs[:, t * STN:(t + 1) * STN], [k] * 32)
                            bcv[(t, k)] = bct
                        continue
                    bct = pBC.tile([C, n, ROWS, W], BF16,
                                   tag=f"bc{n}", bufs=(RING_BC2 if n == 2 else RING_BC1))
                    src = AP(sfd.tensor, ks[0] * HW + t * STN,
                             [[0, C], [HW, n], [1, STN]])
                    dma = nc.sync.dma_start if q == 's' else nc.gpsimd.dma_start
                    dma(out=bct.rearrange("p a b c -> p a (b c)"), in_=src)
                    for j, k in enumerate(ks):
                        bcv[(t, k)] = bct[:, j]

            phase_a(0)

            # ------- mean path (from X2/Y2 directly), readiness order ----
            nc.vector.tensor_scalar(
                out=junk[:, 0:XP_S1 - O_XP], in0=stg[:, O_XP:XP_S1],
                scalar1=0.0, scalar2=1.0, op0=ADD, op1=MULT,
                accum_out=sparts[:, 0:1])
            nc.vector.tensor_reduce(out=rsum[:, 0:1], in_=xpad[:, 1, :],
                                    axis=AX.X, op=ADD)
            nc.vector.tensor_scalar(
                out=junk[:, 0:XP_S2 - XP_S1], in0=stg[:, XP_S1:XP_S2],
                scalar1=0.0, scalar2=1.0, op0=ADD, op1=MULT,
                accum_out=sparts[:, 1:2])
            nc.vector.tensor_scalar(
                out=junk[:, 0:HW // 2], in0=stg[:, O_Y2:Y2_SPLIT],
                scalar1=0.0, scalar2=1.0, op0=ADD, op1=MULT,
                accum_out=y2s[:, 0:1])
            nc.vector.tensor_scalar(
                out=junk[:, 0:HW // 2], in0=stg[:, Y2_SPLIT:PK_COLS],
                scalar1=0.0, scalar2=1.0, op0=ADD, op1=MULT,
                accum_out=y2s[:, 1:2])
            nc.vector.tensor_scalar(
                out=junk[:, 0:O_Y2 - XP_S2], in0=stg[:, XP_S2:O_Y2],
                scalar1=0.0, scalar2=1.0, op0=ADD, op1=MULT,
                accum_out=sparts[:, 2:3])
            nc.vector.tensor_reduce(out=rsum[:, 1:2], in_=xpad[:, 64, :],
                                    axis=AX.X, op=ADD)
            nc.vector.tensor_reduce(out=csum[:, 0:1],
                                    in_=xpad[:, :, 1:2].rearrange("p a b -> p (a b)"),
                                    axis=AX.X, op=ADD)
            nc.vector.tensor_reduce(out=csum[:, 1:2],
                                    in_=xpad[:, :, 64:65].rearrange("p a b -> p (a b)"),
                                    axis=AX.X, op=ADD)
            nc.vector.tensor_reduce(out=stot, in_=sparts[:, 0:3], axis=AX.X, op=ADD)
            # window sums: wsum[c, 3a+b]
            nc.vector.memset(wsum, 0.0)
            nc.vector.tensor_scalar(out=wsum, in0=wsum, scalar1=stot,
                                    scalar2=None, op0=ADD)
            w3d = wsum.rearrange("p (a b) -> p a b", b=3)
            nc.vector.tensor_scalar(out=w3d[:, 0, :], in0=w3d[:, 0, :],
                                    scalar1=rsum[:, 1:2], scalar2=None, op0=SUB)
            nc.vector.tensor_scalar(out=w3d[:, 2, :], in0=w3d[:, 2, :],
                                    scalar1=rsum[:, 0:1], scalar2=None, op0=SUB)
            nc.vector.tensor_scalar(out=w3d[:, :, 0:1].rearrange("p a b -> p (a b)"),
                                    in0=w3d[:, :, 0:1].rearrange("p a b -> p (a b)"),
                                    scalar1=csum[:, 1:2], scalar2=None, op0=SUB)
            nc.vector.tensor_scalar(out=w3d[:, :, 2:3].rearrange("p a b -> p (a b)"),
                                    in0=w3d[:, :, 2:3].rearrange("p a b -> p (a b)"),
                                    scalar1=csum[:, 0:1], scalar2=None, op0=SUB)
            # corner add-backs: (a,b)=(0,0)->X[64,64], (0,2)->X[64,1],
            # (2,0)->X[1,64], (2,2)->X[1,1]  (padded coords)
            for (kk, rr, cc) in ((0, 64, 64), (2, 64, 1), (6, 1, 64), (8, 1, 1)):
                nc.vector.tensor_tensor(out=wsum[:, kk:kk + 1],
                                        in0=wsum[:, kk:kk + 1],
                                        in1=xpad[:, rr, cc:cc + 1], op=ADD)
            nc.vector.tensor_tensor(out=wsum, in0=wsum, in1=swh, op=MULT)
            nc.vector.tensor_reduce(out=mxs, in_=wsum, axis=AX.X, op=ADD)
            nc.vector.tensor_reduce(out=my2, in_=y2s, axis=AX.X, op=ADD)

            # ---------------- context branch (w2@w1 folded on host) -----
            ctxp = psOUT.tile([C, 16], F32, tag="ctx", bufs=1)
            nc.tensor.matmul(ctxp[0:64, 0:1], w21aT, mxs, start=True, stop=False)
            nc.tensor.matmul(ctxp[0:64, 0:1], w21bT, my2, start=False, stop=True)
            nc.scalar.activation(out=ctx2, in_=ctxp[0:64, 0:1], func=ACT_RELU)
            for k in range(NT):
                nc.tensor.matmul(ctxp[:, 2 + k:3 + k],
                                 w3T[:, k * C:(k + 1) * C], ctx2,
                                 start=True, stop=True)
            nc.scalar.copy(out=cfsb, in_=ctxp[:, 2:2 + NT])

            phase_a(1)
            phase_a(2)
            phase_a(3)

            # ---------------- phase C ----------------
            for t in range(NST):
                out_ps = psOUT.tile([C, 2, 512], F32, tag="out")
                for h in range(2):
                    c0 = t * STN + h * 512
                    nc.tensor.matmul(out_ps[:, h], wfaT, xs[:, c0:c0 + 512],
                                     start=True, stop=False)
                if not PREFETCH_BCAST:
                    emit_bcasts(t)
                for k in range(NT):
                    slot = t * NT + k
                    dh, dw = divmod(k, 3)
                    bc = bcv[(t, k)]
                    xv = xpad[:, 16 * t + dh:16 * t + dh + ROWS, dw:dw + W]
                    p_sb = pP.tile([C, ROWS, W], BF16, tag="p")
                    r = PROD[slot]
                    if r == 'v':
                        nc.vector.scalar_tensor_tensor(
                            out=p_sb, in0=bc, scalar=cfsb[:, k:k + 1],
                            in1=xv, op0=ADD, op1=MULT)
                    elif r == 'g':
                        nc.gpsimd.scalar_tensor_tensor(
                            out=p_sb, in0=bc, scalar=cfsb[:, k:k + 1],
                            in1=xv, op0=ADD, op1=MULT)
                    elif r == 'w':
                        bcc = pBCC.tile([C, ROWS, W], BF16, tag="bcc")
                        nc.vector.tensor_scalar(
                            out=bcc, in0=bc, scalar1=cfsb[:, k:k + 1],
                            scalar2=None, op0=ADD)
                        nc.vector.tensor_tensor(out=p_sb, in0=bcc, in1=xv,
                                                op=MULT)
                    else:
                        bcc = pBCC.tile([C, ROWS, W], BF16, tag="bcc")
                        nc.scalar.activation(out=bcc, in_=bc, func=ACT_IDENT,
                                             bias=cfsb[:, k:k + 1])
                        nc.vector.tensor_tensor(out=p_sb, in0=bcc, in1=xv,
                                                op=MULT)
                    pv = p_sb.rearrange("p a b -> p (a b)")
                    for h in range(2):
                        nc.tensor.matmul(out_ps[:, h], wfbT,
                                         pv[:, h * 512:(h + 1) * 512],
                                         start=False,
                                         stop=(k == NT - 1))
                # per-half output copy on the Act queue (keeps the SP queue
                # free for the broadcast stream); the last tile ships each
                # half separately to shorten the tail
                o_sb = pOsb.tile([C, 2, 8, W], BF16, tag="osb")
                for h in range(2):
                    nc.scalar.copy(out=o_sb[:, h], in_=out_ps[:, h].rearrange(
                        "c (r w) -> c r w", w=W))
                nc.scalar.dma_start(
                    out=ob[:, 16 * t:16 * t + 16, :],
                    in_=o_sb.rearrange("c b r w -> c (b r) w"))

    _split_multiwaits(nc)
    return nc


def _bf(a):
    return np.ascontiguousarray(a, dtype=np.float32).astype(ml_dtypes.bfloat16)


def _f32cols(a):
    """f32 [P, n] -> bf16-typed [P, 2n] view of the same bytes."""
    a = np.ascontiguousarray(a, dtype=np.float32)
    return a.view(np.uint16).view(ml_dtypes.bfloat16)


def _prep_weights(static_w, w1, w2, w3, ws, wf):
    f = np.float32
    pk_w = np.zeros((C, O_XP), dtype=ml_dtypes.bfloat16)
    pk_w[:, O_ID:O_ID + C] = _bf(np.eye(C, dtype=f))
    sw = np.ascontiguousarray(static_w.reshape(C, NT), dtype=f) / HW
    pk_w[:, O_SW:O_SW + 18] = _f32cols(sw)
    wsa_rep = np.zeros((C, C), dtype=f)
    wsb_rep = np.zeros((C, C), dtype=f)
    for g in range(4):
        wsa_rep[:, 32 * g:32 * g + NT] = ws[:, :C].T
        wsb_rep[:, 32 * g:32 * g + NT] = ws[:, C:].T
    pk_w[:, O_WSA:O_WSA + C] = _bf(wsa_rep)
    pk_w[:, O_WSB:O_WSB + C] = _bf(wsb_rep)
    pk_w[:, O_WFA:O_WFA + C] = _bf(wf[:, :C].T)
    pk_w[:, O_WFB:O_WFB + C] = _bf(wf[:, C:].T)
    w21 = (w2.astype(f) @ w1.astype(f))           # fold: relu((w2@w1)@mean)
    pk_w[:, O_W21A:O_W21A + 128] = _f32cols(w21[:, :C].T)
    pk_w[:, O_W21B:O_W21B + 128] = _f32cols(w21[:, C:].T / HW)
    w3t = np.ascontiguousarray(
        w3.reshape(C, NT, 64).transpose(2, 1, 0), dtype=f).reshape(64, NT * C)
    pk_w[0:64, O_W3:O_W3 + NT * C] = _bf(w3t)
    return pk_w


def make_in_maps(X2, Y2, static_w, w1, w2, w3, ws, wf):
    pk_w = _prep_weights(
        np.asarray(static_w), np.asarray(w1), np.asarray(w2),
        np.asarray(w3), np.asarray(ws), np.asarray(wf))
    X2 = np.asarray(X2)
    Y2 = np.asarray(Y2)
    xpad_all = np.zeros((B, C, PH, PW), dtype=np.float32)
    xpad_all[:, :, 1:H + 1, 1:W + 1] = X2
    xpad_all = _bf(xpad_all).reshape(B, C, PH * PW)
    y2_all = _bf(Y2.reshape(B, C, HW))
    in_maps = []
    for b in range(B):
        m = {"pk": np.ascontiguousarray(np.concatenate(
            [pk_w, xpad_all[b], y2_all[b]], axis=1))}
        in_maps.append(m)
    return in_maps


def get_nc():
    if "nc" not in _CACHE:
        _CACHE["nc"] = _build_bass()
    return _CACHE["nc"]


def kernel(X2, Y2, static_w, w1, w2, w3, ws, wf):
    nc = get_nc()
    in_maps = make_in_maps(
        np.asarray(X2), np.asarray(Y2), static_w, w1, w2, w3, ws, wf)
    res = run_bass_kernel_spmd(nc, in_maps, core_ids=list(range(B)))
    out = np.stack([r["ob"] for r in res.results]).astype(np.float32)
    return out
